# revision 7
# baseline (speedup 1.0000x reference)
"""Trainium2 Bass kernel for DecoderRNN (embed -> LSTM -> vocab FC).

Strategy (8 NeuronCores, SPMD):
  - Embedding gather, x_proj precompute and the LSTM recurrence are
    replicated on every core (per-step cross-core collectives are
    latency-bound and slower than replicating ~7us/step of matmul).
  - The dominant FC (hs @ fc_W.T, 134 of 160 GFLOP) is sharded along the
    vocab dim: each core gets 4000 rows of fc_W (zero-padded to 4096) and
    produces logits[:, shard]. Host concatenates.
  - Token order is t-major everywhere (bt = t*64 + b).
  - Matmul inputs bf16, fp32 PSUM accumulation; cell state fp32.
  - LSTM gates are computed in place in PSUM (ACT/DVE read+write PSUM).
  - b_ih+b_hh folded into the encoder projection via K-augmentation
    (host appends a ones column to enc and a bias column to W_enc).
"""

import os
import sys

import numpy as np

for _p in ("/opt/trn_rl_repo", "/root/.axon_site/_ro/trn_rl_repo"):
    if os.path.isdir(_p) and _p not in sys.path:
        sys.path.append(_p)

import ml_dtypes
import concourse.bass as bass
import concourse.mybir as mybir
from concourse import bacc
import concourse.tile as tile
from concourse.bass_utils import run_bass_kernel_spmd
from concourse.masks import make_identity

F32 = mybir.dt.float32
BF16 = mybir.dt.bfloat16
I32 = mybir.dt.int32

B, T = 64, 32
E, H, V = 512, 1024, 32000
G4 = 4 * H          # 4096
BT = B * T          # 2048
EA = E + 128        # augmented enc K dim (ones col + zero pad)
NCORES = 8
VL = V // NCORES    # 4000 real vocab rows per core
VLP = 4096          # padded vocab rows per core

Sig = mybir.ActivationFunctionType.Sigmoid
Tanh = mybir.ActivationFunctionType.Tanh

_nc_cache = None


def build_nc() -> bass.Bass:
    nc = bacc.Bacc()

    idx = nc.declare_dram_parameter("idx", [16, 128, 1], I32, isOutput=False)
    enc = nc.declare_dram_parameter("enc", [B, EA], F32, isOutput=False)
    h0 = nc.declare_dram_parameter("h0", [B, H], F32, isOutput=False)
    c0 = nc.declare_dram_parameter("c0", [B, H], F32, isOutput=False)
    emb = nc.declare_dram_parameter("emb", [V, E], F32, isOutput=False)
    wih = nc.declare_dram_parameter("wih", [G4, E], F32, isOutput=False)
    wenc = nc.declare_dram_parameter("wenc", [G4, EA], F32, isOutput=False)
    whh = nc.declare_dram_parameter("whh", [G4, H], F32, isOutput=False)
    fcw = nc.declare_dram_parameter("fcw", [VLP, H], F32, isOutput=False)
    fcb = nc.declare_dram_parameter("fcb", [128, VLP], BF16, isOutput=False)

    logits = nc.declare_dram_parameter("logits", [BT, VLP], F32, isOutput=True)
    hn = nc.declare_dram_parameter("hn", [B, H], F32, isOutput=True)
    cn = nc.declare_dram_parameter("cn", [B, H], F32, isOutput=True)

    with tile.TileContext(nc) as tc:
        with (
            tc.tile_pool(name="dram", bufs=1, space="DRAM") as dr,
            tc.tile_pool(name="constp", bufs=1) as constp,
            tc.tile_pool(name="wres", bufs=1) as wres,      # WhT + enc_exp2
            tc.tile_pool(name="stream1", bufs=1) as st1,    # embT, WwT, fw
            tc.tile_pool(name="stream2", bufs=2) as st2,    # WeT, hsL, hT, casts
            tc.tile_pool(name="xsbp", bufs=1) as xsbp,
            tc.tile_pool(name="xpp", bufs=3) as xpp,
            tc.tile_pool(name="statep", bufs=1) as statep,
            tc.tile_pool(name="fcop", bufs=2) as fcop,
            tc.tile_pool(name="psA", bufs=2, space="PSUM") as psA,
        ):
            # ---- DRAM scratch
            emb_bf = dr.tile([BT, E], BF16)
            wih_bf = dr.tile([G4, E], BF16)
            wenc_bf = dr.tile([G4, EA], BF16)
            whh_bf = dr.tile([G4, H], BF16)
            enc_bf = dr.tile([B, EA], BF16)
            fcw_bf = dr.tile([VLP, H], BF16)
            xpe = dr.tile([T, B, G4], BF16)
            hsT_dram = dr.tile([8, 128, BT], BF16)

            ident64 = constp.tile([64, 64], F32)
            make_identity(nc, ident64[:])

            # ---- cast inputs to bf16 DRAM scratch ([128,512] col-chunks)
            def cast_to_scratch(src_ap, dst_tile, rows, cols):
                for r0 in range(0, rows, 128):
                    rr = min(128, rows - r0)
                    for c0_ in range(0, cols, 512):
                        cc = min(512, cols - c0_)
                        t_f = st2.tile([128, 512], F32, tag="cast_f")
                        nc.sync.dma_start(
                            t_f[:rr, :cc], src_ap[r0 : r0 + rr, c0_ : c0_ + cc]
                        )
                        t_b = st2.tile([128, 512], BF16, tag="cast_b")
                        nc.vector.tensor_copy(t_b[:rr, :cc], t_f[:rr, :cc])
                        nc.sync.dma_start(
                            dst_tile[r0 : r0 + rr, c0_ : c0_ + cc], t_b[:rr, :cc]
                        )

            cast_to_scratch(wih[:], wih_bf, G4, E)
            cast_to_scratch(wenc[:], wenc_bf, G4, EA)
            cast_to_scratch(whh[:], whh_bf, G4, H)
            cast_to_scratch(fcw[:], fcw_bf, VLP, H)
            cast_to_scratch(enc[:], enc_bf, B, EA)

            # ---- gather embeddings (t-major), cast bf16 -> emb_bf
            for m in range(16):
                idx_sb = st2.tile([128, 1], I32, tag="idx")
                nc.sync.dma_start(idx_sb[:], idx[m])
                g_f = st2.tile([128, E], F32, tag="gf")
                nc.gpsimd.indirect_dma_start(
                    out=g_f[:],
                    out_offset=None,
                    in_=emb[:],
                    in_offset=bass.IndirectOffsetOnAxis(ap=idx_sb[:, :1], axis=0),
                )
                g_b = st2.tile([128, E], BF16, tag="gb")
                nc.vector.tensor_copy(g_b[:], g_f[:])
                nc.sync.dma_start(emb_bf[m * 128 : (m + 1) * 128], g_b[:])

            # ---- resident transposed operands
            embT = []
            for k in range(4):
                tT = st1.tile([128, BT], BF16, tag=f"embT{k}", name=f"embT{k}")
                nc.sync.dma_start_transpose(tT[:], emb_bf[:, k * 128 : (k + 1) * 128])
                embT.append(tT)
            WhT = []
            for k in range(8):
                tT = wres.tile([128, G4], BF16, tag=f"WhT{k}", name=f"WhT{k}")
                nc.sync.dma_start_transpose(tT[:], whh_bf[:, k * 128 : (k + 1) * 128])
                WhT.append(tT)
            encT = []
            for k in range(5):
                tT = st2.tile([128, B], BF16, tag=f"encT{k}", name=f"encT{k}")
                nc.sync.dma_start_transpose(tT[:], enc_bf[:, k * 128 : (k + 1) * 128])
                encT.append(tT)

            # ---- enc_proj (+bias via augmented col) -> enc_exp2 [128,4096] bf16
            enc_exp2 = wres.tile([128, G4], BF16)
            for half in range(2):
                hoff = half * 2048
                ps_e = psA.tile([64, 2048], F32, tag="z")
                for k in range(5):
                    weT = st2.tile([128, 2048], BF16, tag="WeT")
                    nc.sync.dma_start_transpose(
                        weT[:], wenc_bf[hoff : hoff + 2048, k * 128 : (k + 1) * 128]
                    )
                    for n in range(4):
                        nc.tensor.matmul(
                            ps_e[:, n * 512 : (n + 1) * 512],
                            lhsT=encT[k][:],
                            rhs=weT[:, n * 512 : (n + 1) * 512],
                            start=(k == 0),
                            stop=(k == 4),
                        )
                nc.vector.tensor_copy(enc_exp2[0:64, hoff : hoff + 2048], ps_e[:])
            nc.sync.dma_start(enc_exp2[64:128], enc_exp2[0:64])

            # ---- x_proj -> xpe DRAM ([T,B,G4] bf16), includes enc+bias
            for half in range(2):
                hoff = half * 2048
                WwT = []
                for k in range(4):
                    tT = st1.tile([128, 2048], BF16, tag=f"WwT{k}", name=f"WwT{k}")
                    nc.sync.dma_start_transpose(
                        tT[:], wih_bf[hoff : hoff + 2048, k * 128 : (k + 1) * 128]
                    )
                    WwT.append(tT)
                for m in range(16):
                    t0, t1 = 2 * m, 2 * m + 1
                    ps_x = psA.tile([128, 2048], F32, tag="z")
                    for n in range(4):
                        for k in range(4):
                            nc.tensor.matmul(
                                ps_x[:, n * 512 : (n + 1) * 512],
                                lhsT=embT[k][:, m * 128 : (m + 1) * 128],
                                rhs=WwT[k][:, n * 512 : (n + 1) * 512],
                                start=(k == 0),
                                stop=(k == 3),
                            )
                    x_sb = xsbp.tile([128, 2048], BF16, tag="xsb")
                    nc.vector.tensor_add(
                        x_sb[:], ps_x[:], enc_exp2[:, hoff : hoff + 2048]
                    )
                    nc.sync.dma_start(xpe[t0][:, hoff : hoff + 2048], x_sb[0:64])
                    nc.sync.dma_start(xpe[t1][:, hoff : hoff + 2048], x_sb[64:128])

            # ---- LSTM
            h_sb = statep.tile([B, H], F32, tag="h")
            c_sb = statep.tile([B, H], F32, tag="c")
            nc.sync.dma_start(h_sb[:], h0[:])
            nc.sync.dma_start(c_sb[:], c0[:])

            ps_tr0 = psA.tile([128, 512], F32, tag="z")
            for k in range(8):
                nc.tensor.transpose(
                    out=ps_tr0[:, k * 64 : (k + 1) * 64],
                    in_=h_sb[:, k * 128 : (k + 1) * 128],
                    identity=ident64[:],
                )
            hT_prev = st2.tile([128, 512], BF16, tag="hT")
            nc.vector.tensor_copy(hT_prev[:], ps_tr0[:])

            for t in range(T):
                xpA = xpp.tile([B, 2048], BF16, tag="xp")
                nc.sync.dma_start(xpA[:], xpe[t][:, 0:2048])
                xpB = xpp.tile([B, 2048], BF16, tag="xp")
                nc.sync.dma_start(xpB[:], xpe[t][:, 2048:4096])

                zps = []
                for half in range(2):
                    hoff = half * 2048
                    ps_z = psA.tile([64, 2048], F32, tag="z")
                    for n in range(4):
                        for k in range(8):
                            nc.tensor.matmul(
                                ps_z[:, n * 512 : (n + 1) * 512],
                                lhsT=hT_prev[:, k * 64 : (k + 1) * 64],
                                rhs=WhT[k][:, hoff + n * 512 : hoff + (n + 1) * 512],
                                start=(k == 0),
                                stop=(k == 7),
                            )
                    zps.append(ps_z)
                zA, zB = zps  # zA: [i|f], zB: [g|o]

                # DVE ops may read at most ONE input from PSUM: stage sig_i
                # and tanh_c in SBUF, everything else in place in PSUM.
                nc.vector.tensor_add(zA[:], zA[:], xpA[:])
                sigi_sb = statep.tile([B, H], F32, tag="sigi")
                nc.scalar.activation(sigi_sb[:], zA[:, 0:H], Sig)
                nc.scalar.activation(zA[:, H:2048], zA[:, H:2048], Sig)
                nc.vector.tensor_add(zB[:], zB[:], xpB[:])
                nc.scalar.activation(zB[:, 0:H], zB[:, 0:H], Tanh)
                nc.scalar.activation(zB[:, H:2048], zB[:, H:2048], Sig)

                nc.vector.tensor_mul(c_sb[:], zA[:, H:2048], c_sb[:])
                nc.vector.tensor_mul(zB[:, 0:H], sigi_sb[:], zB[:, 0:H])
                nc.vector.tensor_add(c_sb[:], c_sb[:], zB[:, 0:H])
                tc_sb = statep.tile([B, H], F32, tag="tanhc")
                nc.scalar.activation(tc_sb[:], c_sb[:], Tanh)
                nc.vector.tensor_mul(h_sb[:], zB[:, H:2048], tc_sb[:])

                ps_tr = psA.tile([128, 512], F32, tag="z")
                for k in range(8):
                    nc.tensor.transpose(
                        out=ps_tr[:, k * 64 : (k + 1) * 64],
                        in_=h_sb[:, k * 128 : (k + 1) * 128],
                        identity=ident64[:],
                    )
                hT_cur = st2.tile([128, 512], BF16, tag="hT")
                nc.vector.tensor_copy(hT_cur[:], ps_tr[:])
                for k in range(8):
                    nc.sync.dma_start(
                        hsT_dram[k][:, t * 64 : (t + 1) * 64],
                        hT_cur[:, k * 64 : (k + 1) * 64],
                    )
                hT_prev = hT_cur

            nc.sync.dma_start(hn[:], h_sb[:])
            nc.sync.dma_start(cn[:], c_sb[:])

            # ---- FC: logits[bt, v] = hs @ fc_W.T + fc_b (vocab shard)
            fcb_sb = constp.tile([128, VLP], BF16)
            nc.sync.dma_start(fcb_sb[:], fcb[:])
            for vc in range(8):
                v0 = vc * 512
                fw = []
                for k in range(8):
                    fwk = st1.tile([128, 512], BF16, tag=f"fw{k}", name=f"fw{k}")
                    nc.sync.dma_start_transpose(
                        fwk[:], fcw_bf[v0 : v0 + 512, k * 128 : (k + 1) * 128]
                    )
                    fw.append(fwk)
                for m in range(16):
                    hsL = []
                    for k in range(8):
                        hk = st2.tile([128, 128], BF16, tag=f"hsL{k}", name=f"hsL{k}")
                        nc.sync.dma_start(
                            hk[:], hsT_dram[k][:, m * 128 : (m + 1) * 128]
                        )
                        hsL.append(hk)
                    ps_f = psA.tile([128, 512], F32, tag="z")
                    for k in range(8):
                        nc.tensor.matmul(
                            ps_f[:],
                            lhsT=hsL[k][:],
                            rhs=fw[k][:],
                            start=(k == 0),
                            stop=(k == 7),
                        )
                    o_sb = fcop.tile([128, 512], F32, tag="fco")
                    nc.vector.tensor_add(o_sb[:], ps_f[:], fcb_sb[:, v0 : v0 + 512])
                    nc.sync.dma_start(
                        logits[m * 128 : (m + 1) * 128, v0 : v0 + 512], o_sb[:]
                    )

    nc.compile()
    return nc


def get_nc() -> bass.Bass:
    global _nc_cache
    if _nc_cache is None:
        _nc_cache = build_nc()
    return _nc_cache


def make_in_maps(inputs_np: dict) -> list:
    inp = {k: np.asarray(v) for k, v in inputs_np.items()}
    idx_tm = (
        np.ascontiguousarray(inp["inputs"].astype(np.int32).T)
        .reshape(16, 128, 1)
        .copy()
    )
    W_ih = np.asarray(inp["W_ih"], dtype=np.float32)
    bias = (
        np.asarray(inp["b_ih"], dtype=np.float32)
        + np.asarray(inp["b_hh"], dtype=np.float32)
    )
    enc_aug = np.zeros((B, EA), dtype=np.float32)
    enc_aug[:, :E] = np.asarray(inp["encoder_features"], dtype=np.float32)
    enc_aug[:, E] = 1.0
    wenc_aug = np.zeros((G4, EA), dtype=np.float32)
    wenc_aug[:, :E] = W_ih[:, E:]
    wenc_aug[:, E] = bias
    common = dict(
        idx=idx_tm,
        enc=enc_aug,
        h0=np.ascontiguousarray(inp["h0"], dtype=np.float32),
        c0=np.ascontiguousarray(inp["c0"], dtype=np.float32),
        emb=np.ascontiguousarray(inp["embed_table"], dtype=np.float32),
        wih=np.ascontiguousarray(W_ih[:, :E]),
        wenc=wenc_aug,
        whh=np.ascontiguousarray(inp["W_hh"], dtype=np.float32),
    )
    fc_W = np.asarray(inp["fc_W"], dtype=np.float32)
    fc_b = np.asarray(inp["fc_b"], dtype=np.float32)
    in_maps = []
    for i in range(NCORES):
        fcw_pad = np.zeros((VLP, H), dtype=np.float32)
        fcw_pad[:VL] = fc_W[i * VL : (i + 1) * VL]
        fcb_pad = np.zeros((1, VLP), dtype=np.float32)
        fcb_pad[0, :VL] = fc_b[i * VL : (i + 1) * VL]
        fcb_rep = np.broadcast_to(fcb_pad, (128, VLP)).astype(ml_dtypes.bfloat16)
        in_maps.append(dict(common, fcw=fcw_pad, fcb=np.ascontiguousarray(fcb_rep)))
    return in_maps


def assemble(results: list):
    logits_tm = np.concatenate(
        [results[i]["logits"][:, :VL] for i in range(NCORES)], axis=1
    )  # [BT(t-major), V]
    logits = np.ascontiguousarray(
        logits_tm.reshape(T, B, V).transpose(1, 0, 2)
    )  # [B, T, V]
    hn = results[0]["hn"]
    cn = results[0]["cn"]
    return logits, hn, cn


def run(inputs_np: dict, trace: bool = False):
    nc = get_nc()
    in_maps = make_in_maps(inputs_np)
    res = run_bass_kernel_spmd(
        nc, in_maps, core_ids=list(range(NCORES)), trace=trace
    )
    return assemble(res.results), res


def kernel(**inputs) -> tuple:
    (logits, hn, cn), _ = run(inputs, trace=False)
    return logits, hn, cn


# revision 8
# speedup vs baseline: 1.1325x; 1.1325x over previous
"""Trainium2 Bass kernel for DecoderRNN (embed -> LSTM -> vocab FC).

Strategy (8 NeuronCores, SPMD):
  - Embedding gather, x_proj precompute and the LSTM recurrence are
    replicated on every core (per-step cross-core collectives are
    latency-bound and slower than replicating ~7us/step of matmul).
  - The dominant FC (hs @ fc_W.T, 134 of 160 GFLOP) is sharded along the
    vocab dim: each core gets 4000 rows of fc_W (zero-padded to 4096) and
    produces logits[:, shard]. Host concatenates.
  - Token order is t-major everywhere (bt = t*64 + b).
  - Matmul inputs bf16, fp32 PSUM accumulation; cell state fp32.
  - Two vocab chunks of the FC are interleaved into the LSTM loop (one
    [128bt x 512v] piece per step) to keep the PE busy through the
    gate-computation latency of the recurrence; the rest runs after.
  - b_ih+b_hh folded into the encoder projection via K-augmentation
    (host appends a ones column to enc and a bias column to W_enc).
"""

import os
import sys

import numpy as np

for _p in ("/opt/trn_rl_repo", "/root/.axon_site/_ro/trn_rl_repo"):
    if os.path.isdir(_p) and _p not in sys.path:
        sys.path.append(_p)

import ml_dtypes
import concourse.bass as bass
import concourse.mybir as mybir
from concourse import bacc
import concourse.tile as tile
from concourse.bass_utils import run_bass_kernel_spmd
from concourse.masks import make_identity

F32 = mybir.dt.float32
BF16 = mybir.dt.bfloat16
I32 = mybir.dt.int32

B, T = 64, 32
E, H, V = 512, 1024, 32000
G4 = 4 * H          # 4096
BT = B * T          # 2048
EA = E + 128        # augmented enc K dim (ones col + zero pad)
NCORES = 8
VL = V // NCORES    # 4000 real vocab rows per core
VLP = 4096          # padded vocab rows per core

Sig = mybir.ActivationFunctionType.Sigmoid
Tanh = mybir.ActivationFunctionType.Tanh

_nc_cache = None


def build_nc() -> bass.Bass:
    nc = bacc.Bacc()

    idx = nc.declare_dram_parameter("idx", [16, 128, 1], I32, isOutput=False)
    enc = nc.declare_dram_parameter("enc", [B, EA], F32, isOutput=False)
    h0 = nc.declare_dram_parameter("h0", [B, H], F32, isOutput=False)
    c0 = nc.declare_dram_parameter("c0", [B, H], F32, isOutput=False)
    emb = nc.declare_dram_parameter("emb", [V, E], F32, isOutput=False)
    wih = nc.declare_dram_parameter("wih", [G4, E], F32, isOutput=False)
    wenc = nc.declare_dram_parameter("wenc", [G4, EA], F32, isOutput=False)
    whh = nc.declare_dram_parameter("whh", [G4, H], F32, isOutput=False)
    fcw = nc.declare_dram_parameter("fcw", [VLP, H], F32, isOutput=False)
    fcb = nc.declare_dram_parameter("fcb", [128, VLP], BF16, isOutput=False)

    logits = nc.declare_dram_parameter("logits", [BT, VLP], F32, isOutput=True)
    hn = nc.declare_dram_parameter("hn", [B, H], F32, isOutput=True)
    cn = nc.declare_dram_parameter("cn", [B, H], F32, isOutput=True)

    with tile.TileContext(nc) as tc:
        with (
            tc.tile_pool(name="dram", bufs=1, space="DRAM") as dr,
            tc.tile_pool(name="constp", bufs=1) as constp,
            tc.tile_pool(name="wres", bufs=1) as wres,
            tc.tile_pool(name="stream1", bufs=1) as st1,
            tc.tile_pool(name="stream2", bufs=2) as st2,
            tc.tile_pool(name="xsbp", bufs=2) as xsbp,
            tc.tile_pool(name="xpp", bufs=3) as xpp,
            tc.tile_pool(name="statep", bufs=1) as statep,
            tc.tile_pool(name="fcop", bufs=2) as fcop,
            tc.tile_pool(name="psA", bufs=2, space="PSUM") as psA,
        ):
            # ---- DRAM scratch
            emb_bf = dr.tile([BT, E], BF16)
            wih_bf = dr.tile([G4, E], BF16)
            wenc_bf = dr.tile([G4, EA], BF16)
            whh_bf = dr.tile([G4, H], BF16)
            enc_bf = dr.tile([B, EA], BF16)
            fcw_bf = dr.tile([VLP, H], BF16)
            xpe = dr.tile([T, B, G4], BF16)
            # one tile per bt-block so the interleaved FC's RAW dep is only
            # on the two steps that wrote the block
            hsT_d = [
                dr.tile([8, 128, 128], BF16, name=f"hsT_d{m}") for m in range(16)
            ]

            ident64 = constp.tile([64, 64], F32)
            make_identity(nc, ident64[:])

            def cast_to_scratch(src_ap, dst_tile, rows, cols, eng):
                for r0 in range(0, rows, 128):
                    rr = min(128, rows - r0)
                    for cx in range(0, cols, 512):
                        cc = min(512, cols - cx)
                        t_f = st2.tile([128, 512], F32, tag="cast_f")
                        eng.dma_start(
                            t_f[:rr, :cc], src_ap[r0 : r0 + rr, cx : cx + cc]
                        )
                        t_b = st2.tile([128, 512], BF16, tag="cast_b")
                        nc.vector.tensor_copy(t_b[:rr, :cc], t_f[:rr, :cc])
                        eng.dma_start(
                            dst_tile[r0 : r0 + rr, cx : cx + cc], t_b[:rr, :cc]
                        )

            # ---- gather embeddings first (t-major), cast bf16 -> emb_bf
            for m in range(16):
                idx_sb = st2.tile([128, 1], I32, tag="idx")
                nc.sync.dma_start(idx_sb[:], idx[m])
                g_f = st2.tile([128, E], F32, tag="gf")
                nc.gpsimd.indirect_dma_start(
                    out=g_f[:],
                    out_offset=None,
                    in_=emb[:],
                    in_offset=bass.IndirectOffsetOnAxis(ap=idx_sb[:, :1], axis=0),
                )
                g_b = st2.tile([128, E], BF16, tag="gb")
                nc.vector.tensor_copy(g_b[:], g_f[:])
                nc.sync.dma_start(emb_bf[m * 128 : (m + 1) * 128], g_b[:])

            # embT needed for the very first matmuls
            embT = []
            for k in range(4):
                tT = st1.tile([128, BT], BF16, tag=f"embT{k}", name=f"embT{k}")
                nc.sync.dma_start_transpose(tT[:], emb_bf[:, k * 128 : (k + 1) * 128])
                embT.append(tT)

            cast_to_scratch(wih[:], wih_bf, G4, E, nc.sync)
            cast_to_scratch(enc[:], enc_bf, B, EA, nc.sync)
            cast_to_scratch(wenc[:], wenc_bf, G4, EA, nc.scalar)

            encT = []
            for k in range(5):
                tT = st2.tile([128, B], BF16, tag=f"encT{k}", name=f"encT{k}")
                nc.sync.dma_start_transpose(tT[:], enc_bf[:, k * 128 : (k + 1) * 128])
                encT.append(tT)

            # ---- enc_proj (+bias) -> enc_exp2 [128,4096] bf16 (rows repeated)
            enc_exp2 = wres.tile([128, G4], BF16)
            for half in range(2):
                hoff = half * 2048
                ps_e = psA.tile([64, 2048], F32, tag="z")
                for k in range(5):
                    weT = st2.tile([128, 2048], BF16, tag="WeT")
                    nc.scalar.dma_start_transpose(
                        weT[:], wenc_bf[hoff : hoff + 2048, k * 128 : (k + 1) * 128]
                    )
                    for n in range(4):
                        nc.tensor.matmul(
                            ps_e[:, n * 512 : (n + 1) * 512],
                            lhsT=encT[k][:],
                            rhs=weT[:, n * 512 : (n + 1) * 512],
                            start=(k == 0),
                            stop=(k == 4),
                        )
                nc.vector.tensor_copy(enc_exp2[0:64, hoff : hoff + 2048], ps_e[:])
            nc.sync.dma_start(enc_exp2[64:128], enc_exp2[0:64])

            # ---- x_proj -> xpe DRAM, includes enc+bias
            for half in range(2):
                hoff = half * 2048
                WwT = []
                for k in range(4):
                    tT = st1.tile([128, 2048], BF16, tag=f"WwT{k}", name=f"WwT{k}")
                    nc.sync.dma_start_transpose(
                        tT[:], wih_bf[hoff : hoff + 2048, k * 128 : (k + 1) * 128]
                    )
                    WwT.append(tT)
                for m in range(16):
                    t0, t1 = 2 * m, 2 * m + 1
                    ps_x = psA.tile([128, 2048], F32, tag="z")
                    for n in range(4):
                        for k in range(4):
                            nc.tensor.matmul(
                                ps_x[:, n * 512 : (n + 1) * 512],
                                lhsT=embT[k][:, m * 128 : (m + 1) * 128],
                                rhs=WwT[k][:, n * 512 : (n + 1) * 512],
                                start=(k == 0),
                                stop=(k == 3),
                            )
                    x_sb = xsbp.tile([128, 2048], BF16, tag="xsb")
                    nc.vector.tensor_add(
                        x_sb[:], ps_x[:], enc_exp2[:, hoff : hoff + 2048]
                    )
                    nc.scalar.dma_start(xpe[t0][:, hoff : hoff + 2048], x_sb[0:64])
                    nc.scalar.dma_start(xpe[t1][:, hoff : hoff + 2048], x_sb[64:128])

            # whh/fcw casts: needed later; keep off the x_proj critical path
            cast_to_scratch(whh[:], whh_bf, G4, H, nc.sync)
            WhT = []
            for k in range(8):
                tT = wres.tile([128, G4], BF16, tag=f"WhT{k}", name=f"WhT{k}")
                nc.sync.dma_start_transpose(tT[:], whh_bf[:, k * 128 : (k + 1) * 128])
                WhT.append(tT)
            # chunks 0,1 of fcw early (interleaved FC); rest later
            cast_to_scratch(fcw[0:1024], fcw_bf[0:1024], 1024, H, nc.scalar)

            # fw tiles for the two interleaved vocab chunks (resident)
            fwI = []
            for vc in range(2):
                row = []
                for k in range(8):
                    fwk = st1.tile(
                        [128, 512], BF16, tag=f"fwI{vc}_{k}", name=f"fwI{vc}_{k}"
                    )
                    nc.scalar.dma_start_transpose(
                        fwk[:],
                        fcw_bf[vc * 512 : (vc + 1) * 512, k * 128 : (k + 1) * 128],
                    )
                    row.append(fwk)
                fwI.append(row)
            fcb_sb = constp.tile([128, VLP], BF16)
            nc.scalar.dma_start(fcb_sb[:], fcb[:])

            # ---- LSTM (+ interleaved FC pieces)
            h_sb = statep.tile([B, H], F32, tag="h")
            c_sb = statep.tile([B, H], F32, tag="c")
            nc.sync.dma_start(h_sb[:], h0[:])
            nc.sync.dma_start(c_sb[:], c0[:])

            ps_tr0 = psA.tile([128, 512], F32, tag="z")
            for k in range(8):
                nc.tensor.transpose(
                    out=ps_tr0[:, k * 64 : (k + 1) * 64],
                    in_=h_sb[:, k * 128 : (k + 1) * 128],
                    identity=ident64[:],
                )
            hT_prev = st2.tile([128, 512], BF16, tag="hT")
            nc.vector.tensor_copy(hT_prev[:], ps_tr0[:])

            def fc_piece(m, vc, fw_row):
                """One [128bt x 512v] FC output block: load hsT block, 8
                matmuls, bias add, store logits."""
                v0 = vc * 512
                hsL = []
                for k in range(8):
                    hk = st2.tile([128, 128], BF16, tag=f"hsL{k}", name=f"hsL{k}")
                    nc.scalar.dma_start(hk[:], hsT_d[m][k])
                    hsL.append(hk)
                ps_f = psA.tile([128, 512], F32, tag="z")
                for k in range(8):
                    nc.tensor.matmul(
                        ps_f[:],
                        lhsT=hsL[k][:],
                        rhs=fw_row[k][:],
                        start=(k == 0),
                        stop=(k == 7),
                    )
                o_sb = fcop.tile([128, 512], F32, tag="fco")
                nc.vector.tensor_add(o_sb[:], ps_f[:], fcb_sb[:, v0 : v0 + 512])
                nc.scalar.dma_start(
                    logits[m * 128 : (m + 1) * 128, v0 : v0 + 512], o_sb[:]
                )

            for t in range(T):
                xpA = xpp.tile([B, 2048], BF16, tag="xp")
                nc.sync.dma_start(xpA[:], xpe[t][:, 0:2048])
                xpB = xpp.tile([B, 2048], BF16, tag="xp")
                nc.sync.dma_start(xpB[:], xpe[t][:, 2048:4096])

                zps = []
                for half in range(2):
                    hoff = half * 2048
                    ps_z = psA.tile([64, 2048], F32, tag="z")
                    for n in range(4):
                        for k in range(8):
                            nc.tensor.matmul(
                                ps_z[:, n * 512 : (n + 1) * 512],
                                lhsT=hT_prev[:, k * 64 : (k + 1) * 64],
                                rhs=WhT[k][:, hoff + n * 512 : hoff + (n + 1) * 512],
                                start=(k == 0),
                                stop=(k == 7),
                            )
                    zps.append(ps_z)
                zA, zB = zps  # zA: [i|f], zB: [g|o]

                # PE filler while gates compute: one FC piece per step
                if t >= 2:
                    p = t - 2
                    fc_piece(p // 2, p % 2, fwI[p % 2])

                # gates: zA in-place in PSUM; zB lands in SBUF (frees slot)
                nc.vector.tensor_add(zA[:], zA[:], xpA[:])
                nc.scalar.activation(zA[:], zA[:], Sig)  # sig_i | sig_f
                go_sb = statep.tile([B, 2048], F32, tag="go")
                nc.vector.tensor_add(go_sb[:], zB[:], xpB[:])
                nc.scalar.activation(go_sb[:, 0:H], go_sb[:, 0:H], Tanh)
                nc.scalar.activation(go_sb[:, H:2048], go_sb[:, H:2048], Sig)

                nc.vector.tensor_mul(c_sb[:], zA[:, H:2048], c_sb[:])
                nc.vector.tensor_mul(go_sb[:, 0:H], zA[:, 0:H], go_sb[:, 0:H])
                nc.vector.tensor_add(c_sb[:], c_sb[:], go_sb[:, 0:H])
                nc.scalar.activation(go_sb[:, 0:H], c_sb[:], Tanh)  # tanh(c)
                nc.vector.tensor_mul(h_sb[:], go_sb[:, H:2048], go_sb[:, 0:H])

                ps_tr = psA.tile([128, 512], F32, tag="z")
                for k in range(8):
                    nc.tensor.transpose(
                        out=ps_tr[:, k * 64 : (k + 1) * 64],
                        in_=h_sb[:, k * 128 : (k + 1) * 128],
                        identity=ident64[:],
                    )
                hT_cur = st2.tile([128, 512], BF16, tag="hT")
                nc.vector.tensor_copy(hT_cur[:], ps_tr[:])
                m, hlf = t // 2, t % 2
                for k in range(8):
                    nc.sync.dma_start(
                        hsT_d[m][k][:, hlf * 64 : (hlf + 1) * 64],
                        hT_cur[:, k * 64 : (k + 1) * 64],
                    )
                hT_prev = hT_cur

            # leftover interleaved pieces (m=15)
            fc_piece(15, 0, fwI[0])
            fc_piece(15, 1, fwI[1])

            nc.sync.dma_start(hn[:], h_sb[:])
            nc.sync.dma_start(cn[:], c_sb[:])

            # ---- remaining FC chunks 2..7
            cast_to_scratch(fcw[1024:VLP], fcw_bf[1024:VLP], VLP - 1024, H, nc.scalar)
            for vc in range(2, 8):
                v0 = vc * 512
                fw = []
                for k in range(8):
                    fwk = st1.tile(
                        [128, 512], BF16,
                        tag=f"fwI{vc % 2}_{k}", name=f"fw{vc}_{k}",
                    )
                    nc.scalar.dma_start_transpose(
                        fwk[:], fcw_bf[v0 : v0 + 512, k * 128 : (k + 1) * 128]
                    )
                    fw.append(fwk)
                for m in range(16):
                    fc_piece(m, vc, fw)

    nc.compile()
    return nc


def get_nc() -> bass.Bass:
    global _nc_cache
    if _nc_cache is None:
        _nc_cache = build_nc()
    return _nc_cache


def make_in_maps(inputs_np: dict) -> list:
    inp = {k: np.asarray(v) for k, v in inputs_np.items()}
    idx_tm = (
        np.ascontiguousarray(inp["inputs"].astype(np.int32).T)
        .reshape(16, 128, 1)
        .copy()
    )
    W_ih = np.asarray(inp["W_ih"], dtype=np.float32)
    bias = (
        np.asarray(inp["b_ih"], dtype=np.float32)
        + np.asarray(inp["b_hh"], dtype=np.float32)
    )
    enc_aug = np.zeros((B, EA), dtype=np.float32)
    enc_aug[:, :E] = np.asarray(inp["encoder_features"], dtype=np.float32)
    enc_aug[:, E] = 1.0
    wenc_aug = np.zeros((G4, EA), dtype=np.float32)
    wenc_aug[:, :E] = W_ih[:, E:]
    wenc_aug[:, E] = bias
    common = dict(
        idx=idx_tm,
        enc=enc_aug,
        h0=np.ascontiguousarray(inp["h0"], dtype=np.float32),
        c0=np.ascontiguousarray(inp["c0"], dtype=np.float32),
        emb=np.ascontiguousarray(inp["embed_table"], dtype=np.float32),
        wih=np.ascontiguousarray(W_ih[:, :E]),
        wenc=wenc_aug,
        whh=np.ascontiguousarray(inp["W_hh"], dtype=np.float32),
    )
    fc_W = np.asarray(inp["fc_W"], dtype=np.float32)
    fc_b = np.asarray(inp["fc_b"], dtype=np.float32)
    in_maps = []
    for i in range(NCORES):
        fcw_pad = np.zeros((VLP, H), dtype=np.float32)
        fcw_pad[:VL] = fc_W[i * VL : (i + 1) * VL]
        fcb_pad = np.zeros((1, VLP), dtype=np.float32)
        fcb_pad[0, :VL] = fc_b[i * VL : (i + 1) * VL]
        fcb_rep = np.broadcast_to(fcb_pad, (128, VLP)).astype(ml_dtypes.bfloat16)
        in_maps.append(dict(common, fcw=fcw_pad, fcb=np.ascontiguousarray(fcb_rep)))
    return in_maps


def assemble(results: list):
    logits_tm = np.concatenate(
        [results[i]["logits"][:, :VL] for i in range(NCORES)], axis=1
    )  # [BT(t-major), V]
    logits = np.ascontiguousarray(
        logits_tm.reshape(T, B, V).transpose(1, 0, 2)
    )  # [B, T, V]
    hn = results[0]["hn"]
    cn = results[0]["cn"]
    return logits, hn, cn


def run(inputs_np: dict, trace: bool = False):
    nc = get_nc()
    in_maps = make_in_maps(inputs_np)
    res = run_bass_kernel_spmd(
        nc, in_maps, core_ids=list(range(NCORES)), trace=trace
    )
    return assemble(res.results), res


def kernel(**inputs) -> tuple:
    (logits, hn, cn), _ = run(inputs, trace=False)
    return logits, hn, cn
